# revision 16
# baseline (speedup 1.0000x reference)
"""Multi-head self-attention (B=4, L=2048, D=1024, H=16, RoPE, causal) on 8
Trainium2 NeuronCores.

Sharding: data-parallel over batch (4) x tensor-parallel over head groups (2).
Core i handles batch i//2, heads 8*(i%2) .. 8*(i%2)+8.  Each core computes its
QKV projection slice, RoPE, causal attention for its 8 heads, and a partial
output projection over its 512 d-columns; the host sums the two partials per
batch.

On-core dataflow (per core, all matmul operands bf16, psum/softmax fp32):
  qkT[e,l] = Wqk_sub @ x^T       (e = 8 q-heads then 8 k-heads, dh-major)
  rope on qkT rows (pair-swap via stream_shuffle + cos/sin tables)
  V[l,e]   = x @ Wv_sub^T        (natural orientation, 8 heads * 64)
  per head-pair (2 heads stacked in one 128-partition chunk):
    S^T[k,q] = K^T Q  (row-paired K=64 matmuls, tile_position (0,0)/(64,0))
    P^T = exp(S^T/8) with causal masking (valid-range exp + tri-mask)
    s[q] = colsum(P^T) broadcast via ones-matmul (col-paired (0,0)/(0,64))
    O^T[dh,q] = V^T-matmul accumulation (col-paired)
    O^T /= s
  y[l,e] partial = O^T.T @ Wo_sub  (lhsT = O^T chunks)

Scheduling: single fused phase.  Rounds are ordered jq-major (q-block outer,
head-pair inner) and the QKV-projection l4-blocks, V tiles, and output-
projection halves are emitted just-in-time / interleaved between attention
tiles so the tensor engine never idles and exp (scalar engine) overlaps the
projection work.  The exp input is packed (b-half shifted down by vs) so no
garbage columns are exponentiated.
"""
import sys
sys.path.insert(0, "/opt/trn_rl_repo")

import numpy as np
import ml_dtypes

B, L, D, H = 4, 2048, 1024, 16
DH = D // H  # 64
THETA = 100000.0
NCORES = 8
BF = ml_dtypes.bfloat16

_built = None


def _rope_tables():
    # [128, L] f32: rows = 2 stacked heads' dh (64 each), identical per head.
    pos = np.arange(L, dtype=np.float32)
    inv_freq = (1.0 / THETA ** (np.arange(0, DH, 2, dtype=np.float32) / DH))
    ang = pos[None, :] * inv_freq[:, None]              # [32, L]
    cos = np.cos(ang)                                    # [32, L]
    sin = np.sin(ang)
    cos2 = np.repeat(cos, 2, axis=0)                     # rows 2p,2p+1 = cos_p
    sin2 = np.empty((DH, L), np.float32)
    sin2[0::2] = -sin
    sin2[1::2] = sin
    return (np.concatenate([cos2, cos2], 0).astype(np.float32),
            np.concatenate([sin2, sin2], 0).astype(np.float32))


def _build():
    import concourse.mybir as mybir
    import concourse.tile as tile
    from concourse import bacc

    FP32 = mybir.dt.float32
    BF16 = mybir.dt.bfloat16
    MUL = mybir.AluOpType.mult
    ADD = mybir.AluOpType.add
    EXP = mybir.ActivationFunctionType.Exp
    SWAP_MASK = [i ^ 1 for i in range(32)]
    SKEW = 3

    nc = bacc.Bacc(None, target_bir_lowering=False)
    # DRAM parameters (per-core shapes; host prepares layouts)
    xt_d = nc.declare_dram_parameter("xt", [8, 128, L], BF16, False)        # x^T d-chunks
    wqk_d = nc.declare_dram_parameter("wqk", [8, 8, 128, 128], BF16, False)  # [dchunk, echunk, d, e]
    wv_d = nc.declare_dram_parameter("wv", [8, 128, 512], BF16, False)      # [dchunk, d, e_v]
    wo_d = nc.declare_dram_parameter("wo", [4, 2, 128, 512], BF16, False)   # [dchunk, ehalf, d, e]
    cos_d = nc.declare_dram_parameter("cos2", [128, L], FP32, False)
    sin_d = nc.declare_dram_parameter("sin2", [128, L], FP32, False)
    tri_d = nc.declare_dram_parameter("trimask", [128, 128], BF16, False)
    y_d = nc.declare_dram_parameter("y", [L, D], FP32, True)

    with tile.TileContext(nc) as tc:
        import contextlib
        ctx = contextlib.ExitStack()
        with ctx:
            # ---- SBUF pools ----
            res = ctx.enter_context(tc.tile_pool(name="res", bufs=1))
            wq_pool = ctx.enter_context(tc.tile_pool(name="wqk", bufs=64))
            rope_pool = ctx.enter_context(tc.tile_pool(name="rope", bufs=3))
            pt_pool = ctx.enter_context(tc.tile_pool(name="pt", bufs=6))
            rec_pool = ctx.enter_context(tc.tile_pool(name="rec", bufs=1))
            y_pool = ctx.enter_context(tc.tile_pool(name="yt", bufs=2))
            # ---- PSUM pools (8 banks total, all open for whole kernel) ----
            st_ps = ctx.enter_context(
                tc.tile_pool(name="ps_st", bufs=2, space="PSUM"))   # 4 banks
            av_ps = ctx.enter_context(
                tc.tile_pool(name="ps_av", bufs=1, space="PSUM"))   # 2 banks
            sh_ps = ctx.enter_context(
                tc.tile_pool(name="ps_sh", bufs=2, space="PSUM"))   # 2 banks

            xt = [res.tile([128, L], BF16, tag=f"xt{d}", name=f"xt{d}")
                  for d in range(8)]
            qkr = [res.tile([128, L], BF16, tag=f"qkr{c}", name=f"qkr{c}")
                   for c in range(8)]
            # V tiles with the softmax-denominator trick: per head h the
            # 128-col block is [ones (64 cols) | V_h (64 cols)], so the AV
            # matmul's output rows 0:64 are the colsum of P^T for free
            # (ones first so the reciprocal reads a base-0 partition range).
            vsb = [res.tile([128, 1024], BF16, tag=f"v{t}", name=f"v{t}")
                   for t in range(16)]
            wv_sb = [res.tile([128, 512], BF16, tag=f"wv{d}", name=f"wv{d}")
                     for d in range(8)]
            wo_sb = [res.tile([128, 512], BF16, tag=f"wo{i}", name=f"wo{i}")
                     for i in range(8)]
            cos_sb = res.tile([128, L], FP32, tag="cos")
            sin_sb = res.tile([128, L], FP32, tag="sin")
            tri_sb = res.tile([128, 128], BF16, tag="tri")
            ones_sb = res.tile([128, DH], BF16, tag="ones")
            wsc_sb = res.tile([128, 512], BF16, tag="wsc")
            ot = [res.tile([128, L], BF16, tag=f"ot{p}", name=f"ot{p}")
                  for p in range(4)]

            # ---- PE warmup: ~4.5us of matmuls with no DMA dependency so the
            # HAM clock-gate opens while the prologue DMAs are in flight ----
            nc.vector.memset(ones_sb, 1.0)
            nc.vector.memset(wsc_sb, 0.0)
            wu = av_ps.tile([128, 512], FP32, tag="ava", name="wu")
            for i in range(40):
                nc.tensor.matmul(wu[0:64, 0:128], ones_sb, wsc_sb[:, 0:128],
                                 start=True, stop=True)

            # ---- JIT DMA/emission state ----
            w_chunks = {}
            xt_q = set()
            cs_q = set()
            blocks_done = set()
            v_done = set()
            wo_loaded = [False]

            def ensure_xt(l4):
                if l4 in xt_q:
                    return
                xt_q.add(l4)
                lsl = slice(512 * l4, 512 * l4 + 512)
                for d in range(8):
                    nc.sync.dma_start(out=xt[d][:, lsl], in_=xt_d[d][:, lsl])

            def ensure_cs(l4):
                if l4 in cs_q:
                    return
                cs_q.add(l4)
                lsl = slice(512 * l4, 512 * l4 + 512)
                nc.sync.dma_start(out=cos_sb[:, lsl], in_=cos_d[:, lsl])
                nc.sync.dma_start(out=sin_sb[:, lsl], in_=sin_d[:, lsl])

            def ensure_w(c):
                if c not in w_chunks:
                    wts = []
                    for d in range(8):
                        w = wq_pool.tile([128, 128], BF16, tag="w",
                                         name=f"w_{c}_{d}")
                        nc.sync.dma_start(out=w, in_=wqk_d[d, c])
                        wts.append(w)
                    w_chunks[c] = wts
                return w_chunks[c]

            def emit_block(c, l4):
                """One 512-wide l-block of QKV-projection chunk c (+RoPE)."""
                if (c, l4) in blocks_done:
                    return
                blocks_done.add((c, l4))
                wts = ensure_w(c)
                ensure_xt(l4)
                ensure_cs(l4)
                lsl = slice(512 * l4, 512 * l4 + 512)
                qkp = sh_ps.tile([128, 512], FP32, tag="ps")
                for d in range(8):
                    nc.tensor.matmul(qkp, wts[d], xt[d][:, lsl],
                                     start=(d == 0), stop=(d == 7))
                shf = rope_pool.tile([128, 512], FP32, tag="shf")
                nc.vector.stream_shuffle(shf, qkp, SWAP_MASK)
                t1 = rope_pool.tile([128, 512], FP32, tag="t1")
                nc.vector.tensor_tensor(out=t1, in0=qkp, in1=cos_sb[:, lsl],
                                        op=MUL)
                t2 = rope_pool.tile([128, 512], FP32, tag="t2")
                nc.vector.tensor_tensor(out=t2, in0=shf, in1=sin_sb[:, lsl],
                                        op=MUL)
                nc.gpsimd.tensor_tensor(out=qkr[c][:, lsl], in0=t1, in1=t2,
                                        op=ADD)

            def emit_v(t):
                if t in v_done:
                    return
                v_done.add(t)
                ensure_xt(t // 4)
                vp = sh_ps.tile([128, 512], FP32, tag="ps")
                lsl = slice(128 * t, 128 * t + 128)
                for d in range(8):
                    nc.tensor.matmul(vp, xt[d][:, lsl], wv_sb[d],
                                     start=(d == 0), stop=(d == 7))
                # scatter the 8 heads' 64 V columns into the [V_h | ones]
                # blocks (ones halves were memset in the prologue)
                dst = vsb[t][:, 0:1024].rearrange(
                    "p (h c) -> p h c", c=128)[:, :, 64:128]
                src = vp[:, 0:512].rearrange("p (h c) -> p h c", c=64)
                nc.vector.tensor_copy(out=dst, in_=src)

            def emit_proj(t, eh):
                lsl = slice(128 * t, 128 * t + 128)
                yp = sh_ps.tile([128, 512], FP32, tag="ps")
                for dc in range(4):
                    nc.tensor.matmul(yp, ot[dc][:, lsl], wo_sb[dc * 2 + eh],
                                     start=(dc == 0), stop=(dc == 3))
                yt = y_pool.tile([128, 512], FP32, tag="yt")
                nc.vector.tensor_copy(out=yt, in_=yp)
                nc.sync.dma_start(out=y_d[lsl, 512 * eh:512 * eh + 512],
                                  in_=yt)

            def emit_wo():
                if wo_loaded[0]:
                    return
                wo_loaded[0] = True
                for dc in range(4):
                    for eh in range(2):
                        nc.sync.dma_start(out=wo_sb[dc * 2 + eh],
                                          in_=wo_d[dc, eh])

            # ---- prologue DMAs + first round's blocks ----
            # interleave w-chunk-0 and xt-quarter-0 DMAs per d so the first
            # projection matmul only waits on the first DMA pair
            wts0 = []
            for d in range(8):
                w = wq_pool.tile([128, 128], BF16, tag="w", name=f"w_0_{d}")
                nc.sync.dma_start(out=w, in_=wqk_d[d, 0])
                wts0.append(w)
                nc.sync.dma_start(out=xt[d][:, 0:512], in_=xt_d[d][:, 0:512])
            w_chunks[0] = wts0
            xt_q.add(0)
            emit_block(0, 0)
            emit_block(4, 0)
            nc.sync.dma_start(out=tri_sb, in_=tri_d[:, :])
            for d in range(8):
                nc.sync.dma_start(out=wv_sb[d], in_=wv_d[d])
            for t in range(16):
                nc.vector.memset(vsb[t], 1.0)

            # ---- fused main loop: attention rounds, jq-major ----
            rounds = [(jq, p) for jq in range(4) for p in range(4)]
            proj_queue = []
            pending_norm = [None]

            def emit_norm(p, qb0, ava, avb, n):
                rsa = rec_pool.tile([64, 512], FP32, tag="rsa", name=f"ra{n}")
                sca = rec_pool.tile([64, 512], FP32, tag="sca", name=f"sa{n}")
                rsb = rec_pool.tile([64, 512], FP32, tag="rsb", name=f"rb{n}")
                scb = rec_pool.tile([64, 512], FP32, tag="scb", name=f"sb{n}")
                nc.vector.reciprocal_approx_accurate(
                    out=rsa, in_=ava[0:64], scratch=sca)
                nc.vector.reciprocal_approx_accurate(
                    out=rsb, in_=avb[0:64], scratch=scb)
                nc.vector.tensor_tensor(out=ot[p][0:64, qb0:qb0 + 512],
                                        in0=ava[64:128], in1=rsa, op=MUL)
                nc.vector.tensor_tensor(out=ot[p][64:128, qb0:qb0 + 512],
                                        in0=avb[64:128], in1=rsb, op=MUL)

            for n, (jq, p) in enumerate(rounds):
                qb0 = 512 * jq
                nk = 4 * (jq + 1)
                emit_block(p, jq)
                emit_block(4 + p, jq)
                ava = av_ps.tile([128, 512], FP32, tag="ava", name=f"ava{n}")
                avb = av_ps.tile([128, 512], FP32, tag="avb", name=f"avb{n}")
                qt, kt = qkr[p], qkr[4 + p]
                vca, vcb = 256 * p, 256 * p + 128

                def sav(k, pt, vs):
                    first, last = (k == 0), (k == nk - 1)
                    isl = slice(vs, 512)
                    bsl = slice(512, 1024 - vs)
                    nc.tensor.matmul(ava[:, isl],
                                     vsb[k][:, vca:vca + 128], pt[:, isl],
                                     start=first, stop=last,
                                     skip_group_check=True)
                    nc.tensor.matmul(avb[:, isl],
                                     vsb[k][:, vcb:vcb + 128], pt[:, bsl],
                                     start=first, stop=last,
                                     skip_group_check=True)

                if n + 1 < len(rounds):
                    jq2, p2 = rounds[n + 1]
                    next_blocks = [(p2, jq2), (4 + p2, jq2)]
                else:
                    next_blocks = []

                pending = []
                for k in range(nk):
                    # interleaved filler work (before S^T so an ACT-lag stall
                    # on the st ring doesn't starve the tensor engine)
                    if k == 1 and pending_norm[0] is not None:
                        pending_norm[0]()
                        pending_norm[0] = None
                    if p == 0 and k < 4:
                        emit_v(4 * jq + k)
                    if k == 1 and next_blocks:
                        emit_block(*next_blocks[0])
                    if k == 3 and next_blocks:
                        emit_block(*next_blocks[1])
                    if jq == 0 and p == 3 and k == 3:
                        emit_wo()
                    if k % 4 == 3 and proj_queue:
                        emit_proj(*proj_queue.pop(0))
                    kpos = 128 * k
                    vs = max(0, kpos - qb0)
                    st = st_ps.tile([128, 1024], FP32, tag="st")
                    ksl = slice(kpos, kpos + 128)
                    qsl = slice(qb0 + vs, qb0 + 512)
                    nc.tensor.matmul(st[:, vs:512], kt[0:64, ksl],
                                     qt[0:64, qsl], start=True, stop=True,
                                     tile_position=(0, 0))
                    nc.tensor.matmul(st[:, 512:1024 - vs], kt[64:128, ksl],
                                     qt[64:128, qsl], start=True, stop=True,
                                     tile_position=(64, 0))
                    pt = pt_pool.tile([128, 1024], BF16, tag="pt")
                    nc.scalar.activation(out=pt[:, vs:1024 - vs],
                                         in_=st[:, vs:1024 - vs],
                                         func=EXP, scale=0.125)
                    if kpos >= qb0:
                        nc.vector.tensor_tensor(out=pt[:, vs:vs + 128],
                                                in0=pt[:, vs:vs + 128],
                                                in1=tri_sb, op=MUL)
                        nc.vector.tensor_tensor(out=pt[:, 512:640],
                                                in0=pt[:, 512:640],
                                                in1=tri_sb, op=MUL)
                    pending.append((k, pt, vs))
                    if len(pending) > SKEW:
                        sav(*pending.pop(0))
                for args in pending:
                    sav(*args)
                pending_norm[0] = (lambda p=p, qb0=qb0, ava=ava, avb=avb, n=n:
                                   emit_norm(p, qb0, ava, avb, n))
                if p == 3:
                    for t in range(4 * jq, 4 * jq + 4):
                        for eh in range(2):
                            proj_queue.append((t, eh))

            pending_norm[0]()
            pending_norm[0] = None
            for t, eh in proj_queue:
                emit_proj(t, eh)
    nc.compile()
    return nc


def _get_nc():
    global _built
    if _built is None:
        _built = _build()
    return _built


def _in_maps(x, W, Wo):
    x = np.asarray(x, np.float32)
    W = np.asarray(W, np.float32)
    Wo = np.asarray(Wo, np.float32)

    cos2, sin2 = _rope_tables()
    tri = np.zeros((128, 128), np.float32)
    p_idx = np.arange(128)
    tri[p_idx[:, None] <= p_idx[None, :]] = 1.0  # valid: k <= q
    tri = tri.astype(BF)

    in_maps = []
    for core in range(NCORES):
        b, g = core // 2, core % 2
        xt = np.ascontiguousarray(x[b].T).astype(BF)                # [D, L]
        xt = xt.reshape(8, 128, L)
        wq = W[512 * g:512 * g + 512]                                # [512, D]
        wk = W[D + 512 * g:D + 512 * g + 512]
        wv = W[2 * D + 512 * g:2 * D + 512 * g + 512]
        wqk_t = np.ascontiguousarray(
            np.concatenate([wq, wk], 0).T).astype(BF)                # [D, 1024]
        # -> [dchunk, echunk, 128, 128]
        wqk_t = wqk_t.reshape(8, 128, 8, 128).transpose(0, 2, 1, 3)
        wqk_t = np.ascontiguousarray(wqk_t)
        wv_t = np.ascontiguousarray(wv.T).astype(BF).reshape(8, 128, 512)
        wo_t = np.ascontiguousarray(Wo[:, 512 * g:512 * g + 512].T).astype(BF)  # [512, D]
        wo_t = wo_t.reshape(4, 128, 2, 512).transpose(0, 2, 1, 3)
        wo_t = np.ascontiguousarray(wo_t)
        in_maps.append({
            "xt": xt, "wqk": wqk_t, "wv": wv_t, "wo": wo_t,
            "cos2": cos2, "sin2": sin2, "trimask": tri,
        })
    return in_maps


def kernel(x, W, Wo):
    from concourse.bass_utils import run_bass_kernel_spmd

    res = run_bass_kernel_spmd(_get_nc(), _in_maps(x, W, Wo),
                               list(range(NCORES)))
    out = np.empty((B, L, D), np.float32)
    for b in range(B):
        out[b] = res.results[2 * b]["y"] + res.results[2 * b + 1]["y"]
    return out


def _install_ntff_hook_shim():
    """The trimmed repo lacks antenv.axon_hooks; reconstruct it so
    run_bass_kernel_spmd(trace=True) can NTFF-profile through axon."""
    import sys as _sys, types
    if "antenv.axon_hooks" in _sys.modules:
        return
    import antenv  # noqa: F401
    from trn_agent_boot.trn_boot import _ntff_profile_via_ctypes
    hook = _ntff_profile_via_ctypes("/opt/axon/libaxon_pjrt.so")
    mod = types.ModuleType("antenv.axon_hooks")
    mod.set_axon_ntff_profile_hook = lambda h: None
    mod.get_axon_ntff_profile_hook = lambda: hook
    _sys.modules["antenv.axon_hooks"] = mod


def kernel_traced(x, W, Wo, tmpdir=None):
    """Run with NTFF tracing; returns BassKernelResults (trace in tmpdir)."""
    from concourse.bass_utils import run_bass_kernel_spmd

    _install_ntff_hook_shim()
    res = run_bass_kernel_spmd(_get_nc(), _in_maps(x, W, Wo),
                               list(range(NCORES)), trace=True, tmpdir=tmpdir)
    return res.exec_time_ns


# revision 17
# speedup vs baseline: 1.0018x; 1.0018x over previous
"""Multi-head self-attention (B=4, L=2048, D=1024, H=16, RoPE, causal) on 8
Trainium2 NeuronCores.

Sharding: data-parallel over batch (4) x tensor-parallel over head groups (2).
Core i handles batch i//2, heads 8*(i%2) .. 8*(i%2)+8.  Each core computes its
QKV projection slice, RoPE, causal attention for its 8 heads, and a partial
output projection over its 512 d-columns; the host sums the two partials per
batch.

On-core dataflow (per core, all matmul operands bf16, psum/softmax fp32):
  qkT[e,l] = Wqk_sub @ x^T       (e = 8 q-heads then 8 k-heads, dh-major)
  rope on qkT rows (pair-swap via stream_shuffle + cos/sin tables)
  V[l,e]   = x @ Wv_sub^T        (natural orientation, 8 heads * 64)
  per head-pair (2 heads stacked in one 128-partition chunk):
    S^T[k,q] = K^T Q  (row-paired K=64 matmuls, tile_position (0,0)/(64,0))
    P^T = exp(S^T/8) with causal masking (valid-range exp + tri-mask)
    s[q] = colsum(P^T) broadcast via ones-matmul (col-paired (0,0)/(0,64))
    O^T[dh,q] = V^T-matmul accumulation (col-paired)
    O^T /= s
  y[l,e] partial = O^T.T @ Wo_sub  (lhsT = O^T chunks)

Scheduling: single fused phase.  Rounds are ordered jq-major (q-block outer,
head-pair inner) and the QKV-projection l4-blocks, V tiles, and output-
projection halves are emitted just-in-time / interleaved between attention
tiles so the tensor engine never idles and exp (scalar engine) overlaps the
projection work.  The exp input is packed (b-half shifted down by vs) so no
garbage columns are exponentiated.
"""
import sys
sys.path.insert(0, "/opt/trn_rl_repo")

import numpy as np
import ml_dtypes

B, L, D, H = 4, 2048, 1024, 16
DH = D // H  # 64
THETA = 100000.0
NCORES = 8
BF = ml_dtypes.bfloat16

_built = None


def _rope_tables():
    # [128, L] f32: rows = 2 stacked heads' dh (64 each), identical per head.
    pos = np.arange(L, dtype=np.float32)
    inv_freq = (1.0 / THETA ** (np.arange(0, DH, 2, dtype=np.float32) / DH))
    ang = pos[None, :] * inv_freq[:, None]              # [32, L]
    cos = np.cos(ang)                                    # [32, L]
    sin = np.sin(ang)
    cos2 = np.repeat(cos, 2, axis=0)                     # rows 2p,2p+1 = cos_p
    sin2 = np.empty((DH, L), np.float32)
    sin2[0::2] = -sin
    sin2[1::2] = sin
    return (np.concatenate([cos2, cos2], 0).astype(np.float32),
            np.concatenate([sin2, sin2], 0).astype(np.float32))


def _build():
    import concourse.mybir as mybir
    import concourse.tile as tile
    from concourse import bacc

    FP32 = mybir.dt.float32
    BF16 = mybir.dt.bfloat16
    MUL = mybir.AluOpType.mult
    ADD = mybir.AluOpType.add
    EXP = mybir.ActivationFunctionType.Exp
    SWAP_MASK = [i ^ 1 for i in range(32)]
    SKEW = 3

    nc = bacc.Bacc(None, target_bir_lowering=False)
    # DRAM parameters (per-core shapes; host prepares layouts)
    xt_d = nc.declare_dram_parameter("xt", [8, 128, L], BF16, False)        # x^T d-chunks
    wqk_d = nc.declare_dram_parameter("wqk", [8, 8, 128, 128], BF16, False)  # [dchunk, echunk, d, e]
    wv_d = nc.declare_dram_parameter("wv", [8, 128, 512], BF16, False)      # [dchunk, d, e_v]
    wo_d = nc.declare_dram_parameter("wo", [4, 2, 128, 512], BF16, False)   # [dchunk, ehalf, d, e]
    cos_d = nc.declare_dram_parameter("cos2", [128, L], FP32, False)
    sin_d = nc.declare_dram_parameter("sin2", [128, L], FP32, False)
    tri_d = nc.declare_dram_parameter("trimask", [128, 128], BF16, False)
    y_d = nc.declare_dram_parameter("y", [L, D], FP32, True)

    with tile.TileContext(nc) as tc:
        import contextlib
        ctx = contextlib.ExitStack()
        with ctx:
            # ---- SBUF pools ----
            res = ctx.enter_context(tc.tile_pool(name="res", bufs=1))
            wq_pool = ctx.enter_context(tc.tile_pool(name="wqk", bufs=64))
            rope_pool = ctx.enter_context(tc.tile_pool(name="rope", bufs=3))
            pt_pool = ctx.enter_context(tc.tile_pool(name="pt", bufs=6))
            rec_pool = ctx.enter_context(tc.tile_pool(name="rec", bufs=1))
            y_pool = ctx.enter_context(tc.tile_pool(name="yt", bufs=2))
            # ---- PSUM pools (8 banks total, all open for whole kernel) ----
            st_ps = ctx.enter_context(
                tc.tile_pool(name="ps_st", bufs=2, space="PSUM"))   # 4 banks
            av_ps = ctx.enter_context(
                tc.tile_pool(name="ps_av", bufs=1, space="PSUM"))   # 2 banks
            sh_ps = ctx.enter_context(
                tc.tile_pool(name="ps_sh", bufs=2, space="PSUM"))   # 2 banks

            xt = [res.tile([128, L], BF16, tag=f"xt{d}", name=f"xt{d}")
                  for d in range(8)]
            qkr = [res.tile([128, L], BF16, tag=f"qkr{c}", name=f"qkr{c}")
                   for c in range(8)]
            # V tiles with the softmax-denominator trick: per head h the
            # 128-col block is [ones (64 cols) | V_h (64 cols)], so the AV
            # matmul's output rows 0:64 are the colsum of P^T for free
            # (ones first so the reciprocal reads a base-0 partition range).
            vsb = [res.tile([128, 1024], BF16, tag=f"v{t}", name=f"v{t}")
                   for t in range(16)]
            wv_sb = [res.tile([128, 512], BF16, tag=f"wv{d}", name=f"wv{d}")
                     for d in range(8)]
            wo_sb = [res.tile([128, 512], BF16, tag=f"wo{i}", name=f"wo{i}")
                     for i in range(8)]
            cos_sb = res.tile([128, L], FP32, tag="cos")
            sin_sb = res.tile([128, L], FP32, tag="sin")
            tri_sb = res.tile([128, 128], BF16, tag="tri")
            ones_sb = res.tile([128, DH], BF16, tag="ones")
            wsc_sb = res.tile([128, 512], BF16, tag="wsc")
            ot = [res.tile([128, L], BF16, tag=f"ot{p}", name=f"ot{p}")
                  for p in range(4)]

            # ---- PE warmup: ~4.5us of matmuls with no DMA dependency so the
            # HAM clock-gate opens while the prologue DMAs are in flight ----
            nc.vector.memset(ones_sb, 1.0)
            nc.vector.memset(wsc_sb, 0.0)
            wu = av_ps.tile([128, 512], FP32, tag="ava", name="wu")
            for i in range(40):
                nc.tensor.matmul(wu[0:64, 0:128], ones_sb, wsc_sb[:, 0:128],
                                 start=True, stop=True)

            # ---- JIT DMA/emission state ----
            w_chunks = {}
            xt_q = set()
            cs_q = set()
            blocks_done = set()
            v_done = set()
            wo_loaded = [False]

            def ensure_xt(l4):
                if l4 in xt_q:
                    return
                xt_q.add(l4)
                lsl = slice(512 * l4, 512 * l4 + 512)
                for d in range(8):
                    nc.sync.dma_start(out=xt[d][:, lsl], in_=xt_d[d][:, lsl])

            def ensure_cs(l4):
                if l4 in cs_q:
                    return
                cs_q.add(l4)
                lsl = slice(512 * l4, 512 * l4 + 512)
                nc.sync.dma_start(out=cos_sb[:, lsl], in_=cos_d[:, lsl])
                nc.sync.dma_start(out=sin_sb[:, lsl], in_=sin_d[:, lsl])

            def ensure_w(c):
                if c not in w_chunks:
                    wts = []
                    for d in range(8):
                        w = wq_pool.tile([128, 128], BF16, tag="w",
                                         name=f"w_{c}_{d}")
                        nc.sync.dma_start(out=w, in_=wqk_d[d, c])
                        wts.append(w)
                    w_chunks[c] = wts
                return w_chunks[c]

            def emit_block(c, l4):
                """One 512-wide l-block of QKV-projection chunk c (+RoPE)."""
                if (c, l4) in blocks_done:
                    return
                blocks_done.add((c, l4))
                wts = ensure_w(c)
                ensure_xt(l4)
                ensure_cs(l4)
                lsl = slice(512 * l4, 512 * l4 + 512)
                qkp = sh_ps.tile([128, 512], FP32, tag="ps")
                for d in range(8):
                    nc.tensor.matmul(qkp, wts[d], xt[d][:, lsl],
                                     start=(d == 0), stop=(d == 7))
                shf = rope_pool.tile([128, 512], FP32, tag="shf")
                nc.vector.stream_shuffle(shf, qkp, SWAP_MASK)
                t1 = rope_pool.tile([128, 512], FP32, tag="t1")
                nc.vector.tensor_tensor(out=t1, in0=qkp, in1=cos_sb[:, lsl],
                                        op=MUL)
                t2 = rope_pool.tile([128, 512], FP32, tag="t2")
                nc.gpsimd.tensor_tensor(out=t2, in0=shf, in1=sin_sb[:, lsl],
                                        op=MUL)
                nc.gpsimd.tensor_tensor(out=qkr[c][:, lsl], in0=t1, in1=t2,
                                        op=ADD)

            def emit_v(t):
                if t in v_done:
                    return
                v_done.add(t)
                ensure_xt(t // 4)
                vp = sh_ps.tile([128, 512], FP32, tag="ps")
                lsl = slice(128 * t, 128 * t + 128)
                for d in range(8):
                    nc.tensor.matmul(vp, xt[d][:, lsl], wv_sb[d],
                                     start=(d == 0), stop=(d == 7))
                # scatter the 8 heads' 64 V columns into the [V_h | ones]
                # blocks (ones halves were memset in the prologue)
                dst = vsb[t][:, 0:1024].rearrange(
                    "p (h c) -> p h c", c=128)[:, :, 64:128]
                src = vp[:, 0:512].rearrange("p (h c) -> p h c", c=64)
                nc.vector.tensor_copy(out=dst, in_=src)

            def emit_proj(t, eh):
                lsl = slice(128 * t, 128 * t + 128)
                yp = sh_ps.tile([128, 512], FP32, tag="ps")
                for dc in range(4):
                    nc.tensor.matmul(yp, ot[dc][:, lsl], wo_sb[dc * 2 + eh],
                                     start=(dc == 0), stop=(dc == 3))
                yt = y_pool.tile([128, 512], FP32, tag="yt")
                nc.vector.tensor_copy(out=yt, in_=yp)
                nc.sync.dma_start(out=y_d[lsl, 512 * eh:512 * eh + 512],
                                  in_=yt)

            def emit_wo():
                if wo_loaded[0]:
                    return
                wo_loaded[0] = True
                for dc in range(4):
                    for eh in range(2):
                        nc.sync.dma_start(out=wo_sb[dc * 2 + eh],
                                          in_=wo_d[dc, eh])

            # ---- prologue DMAs + first round's blocks ----
            # interleave w-chunk-0 and xt-quarter-0 DMAs per d so the first
            # projection matmul only waits on the first DMA pair
            wts0 = []
            for d in range(8):
                w = wq_pool.tile([128, 128], BF16, tag="w", name=f"w_0_{d}")
                nc.sync.dma_start(out=w, in_=wqk_d[d, 0])
                wts0.append(w)
                nc.sync.dma_start(out=xt[d][:, 0:512], in_=xt_d[d][:, 0:512])
            w_chunks[0] = wts0
            xt_q.add(0)
            emit_block(0, 0)
            emit_block(4, 0)
            nc.sync.dma_start(out=tri_sb, in_=tri_d[:, :])
            for d in range(8):
                nc.sync.dma_start(out=wv_sb[d], in_=wv_d[d])
            for t in range(16):
                ones_half = vsb[t][:, 0:1024].rearrange(
                    "p (h c) -> p h c", c=128)[:, :, 0:64]
                nc.vector.memset(ones_half, 1.0)

            # ---- fused main loop: attention rounds, jq-major ----
            rounds = [(jq, p) for jq in range(4) for p in range(4)]
            proj_queue = []
            pending_norm = [None]

            def emit_norm(p, qb0, ava, avb, n):
                rsa = rec_pool.tile([64, 512], FP32, tag="rsa", name=f"ra{n}")
                rsb = rec_pool.tile([64, 512], FP32, tag="rsb", name=f"rb{n}")
                nc.vector.reciprocal_approx_fast(out=rsa, in_=ava[0:64])
                nc.vector.reciprocal_approx_fast(out=rsb, in_=avb[0:64])
                nc.vector.tensor_tensor(out=ot[p][0:64, qb0:qb0 + 512],
                                        in0=ava[64:128], in1=rsa, op=MUL)
                nc.vector.tensor_tensor(out=ot[p][64:128, qb0:qb0 + 512],
                                        in0=avb[64:128], in1=rsb, op=MUL)

            for n, (jq, p) in enumerate(rounds):
                qb0 = 512 * jq
                nk = 4 * (jq + 1)
                emit_block(p, jq)
                emit_block(4 + p, jq)
                ava = av_ps.tile([128, 512], FP32, tag="ava", name=f"ava{n}")
                avb = av_ps.tile([128, 512], FP32, tag="avb", name=f"avb{n}")
                qt, kt = qkr[p], qkr[4 + p]
                vca, vcb = 256 * p, 256 * p + 128

                def sav(k, pt, vs):
                    first, last = (k == 0), (k == nk - 1)
                    isl = slice(vs, 512)
                    bsl = slice(512, 1024 - vs)
                    nc.tensor.matmul(ava[:, isl],
                                     vsb[k][:, vca:vca + 128], pt[:, isl],
                                     start=first, stop=last,
                                     skip_group_check=True)
                    nc.tensor.matmul(avb[:, isl],
                                     vsb[k][:, vcb:vcb + 128], pt[:, bsl],
                                     start=first, stop=last,
                                     skip_group_check=True)

                if n + 1 < len(rounds):
                    jq2, p2 = rounds[n + 1]
                    next_blocks = [(p2, jq2), (4 + p2, jq2)]
                else:
                    next_blocks = []

                pending = []
                for k in range(nk):
                    # interleaved filler work (before S^T so an ACT-lag stall
                    # on the st ring doesn't starve the tensor engine)
                    if k == 1 and pending_norm[0] is not None:
                        pending_norm[0]()
                        pending_norm[0] = None
                    if p == 0 and k < 4:
                        emit_v(4 * jq + k)
                    if k == 0 and p == 1 and jq < 3:
                        ensure_xt(jq + 1)
                        ensure_cs(jq + 1)
                    if k == 1 and next_blocks:
                        emit_block(*next_blocks[0])
                    if k == 3 and next_blocks:
                        emit_block(*next_blocks[1])
                    if jq == 0 and p == 3 and k == 3:
                        emit_wo()
                    if k % 4 == 3 and proj_queue:
                        emit_proj(*proj_queue.pop(0))
                    kpos = 128 * k
                    vs = max(0, kpos - qb0)
                    st = st_ps.tile([128, 1024], FP32, tag="st")
                    ksl = slice(kpos, kpos + 128)
                    qsl = slice(qb0 + vs, qb0 + 512)
                    nc.tensor.matmul(st[:, vs:512], kt[0:64, ksl],
                                     qt[0:64, qsl], start=True, stop=True,
                                     tile_position=(0, 0))
                    nc.tensor.matmul(st[:, 512:1024 - vs], kt[64:128, ksl],
                                     qt[64:128, qsl], start=True, stop=True,
                                     tile_position=(64, 0))
                    pt = pt_pool.tile([128, 1024], BF16, tag="pt")
                    nc.scalar.activation(out=pt[:, vs:1024 - vs],
                                         in_=st[:, vs:1024 - vs],
                                         func=EXP, scale=0.125)
                    if kpos >= qb0:
                        nc.vector.tensor_tensor(out=pt[:, vs:vs + 128],
                                                in0=pt[:, vs:vs + 128],
                                                in1=tri_sb, op=MUL)
                        nc.vector.tensor_tensor(out=pt[:, 512:640],
                                                in0=pt[:, 512:640],
                                                in1=tri_sb, op=MUL)
                    pending.append((k, pt, vs))
                    if len(pending) > SKEW:
                        sav(*pending.pop(0))
                for args in pending:
                    sav(*args)
                pending_norm[0] = (lambda p=p, qb0=qb0, ava=ava, avb=avb, n=n:
                                   emit_norm(p, qb0, ava, avb, n))
                if p == 3:
                    for t in range(4 * jq, 4 * jq + 4):
                        for eh in range(2):
                            proj_queue.append((t, eh))

            pending_norm[0]()
            pending_norm[0] = None
            for t, eh in proj_queue:
                emit_proj(t, eh)
    nc.compile()
    return nc


def _get_nc():
    global _built
    if _built is None:
        _built = _build()
    return _built


def _in_maps(x, W, Wo):
    x = np.asarray(x, np.float32)
    W = np.asarray(W, np.float32)
    Wo = np.asarray(Wo, np.float32)

    cos2, sin2 = _rope_tables()
    tri = np.zeros((128, 128), np.float32)
    p_idx = np.arange(128)
    tri[p_idx[:, None] <= p_idx[None, :]] = 1.0  # valid: k <= q
    tri = tri.astype(BF)

    in_maps = []
    for core in range(NCORES):
        b, g = core // 2, core % 2
        xt = np.ascontiguousarray(x[b].T).astype(BF)                # [D, L]
        xt = xt.reshape(8, 128, L)
        wq = W[512 * g:512 * g + 512]                                # [512, D]
        wk = W[D + 512 * g:D + 512 * g + 512]
        wv = W[2 * D + 512 * g:2 * D + 512 * g + 512]
        wqk_t = np.ascontiguousarray(
            np.concatenate([wq, wk], 0).T).astype(BF)                # [D, 1024]
        # -> [dchunk, echunk, 128, 128]
        wqk_t = wqk_t.reshape(8, 128, 8, 128).transpose(0, 2, 1, 3)
        wqk_t = np.ascontiguousarray(wqk_t)
        wv_t = np.ascontiguousarray(wv.T).astype(BF).reshape(8, 128, 512)
        wo_t = np.ascontiguousarray(Wo[:, 512 * g:512 * g + 512].T).astype(BF)  # [512, D]
        wo_t = wo_t.reshape(4, 128, 2, 512).transpose(0, 2, 1, 3)
        wo_t = np.ascontiguousarray(wo_t)
        in_maps.append({
            "xt": xt, "wqk": wqk_t, "wv": wv_t, "wo": wo_t,
            "cos2": cos2, "sin2": sin2, "trimask": tri,
        })
    return in_maps


def kernel(x, W, Wo):
    from concourse.bass_utils import run_bass_kernel_spmd

    res = run_bass_kernel_spmd(_get_nc(), _in_maps(x, W, Wo),
                               list(range(NCORES)))
    out = np.empty((B, L, D), np.float32)
    for b in range(B):
        out[b] = res.results[2 * b]["y"] + res.results[2 * b + 1]["y"]
    return out


def _install_ntff_hook_shim():
    """The trimmed repo lacks antenv.axon_hooks; reconstruct it so
    run_bass_kernel_spmd(trace=True) can NTFF-profile through axon."""
    import sys as _sys, types
    if "antenv.axon_hooks" in _sys.modules:
        return
    import antenv  # noqa: F401
    from trn_agent_boot.trn_boot import _ntff_profile_via_ctypes
    hook = _ntff_profile_via_ctypes("/opt/axon/libaxon_pjrt.so")
    mod = types.ModuleType("antenv.axon_hooks")
    mod.set_axon_ntff_profile_hook = lambda h: None
    mod.get_axon_ntff_profile_hook = lambda: hook
    _sys.modules["antenv.axon_hooks"] = mod


def kernel_traced(x, W, Wo, tmpdir=None):
    """Run with NTFF tracing; returns BassKernelResults (trace in tmpdir)."""
    from concourse.bass_utils import run_bass_kernel_spmd

    _install_ntff_hook_shim()
    res = run_bass_kernel_spmd(_get_nc(), _in_maps(x, W, Wo),
                               list(range(NCORES)), trace=True, tmpdir=tmpdir)
    return res.exec_time_ns


# revision 18
# speedup vs baseline: 1.0825x; 1.0805x over previous
"""Multi-head self-attention (B=4, L=2048, D=1024, H=16, RoPE, causal) on 8
Trainium2 NeuronCores.

Sharding: data-parallel over batch (4) x tensor-parallel over head groups (2).
Core i handles batch i//2, heads 8*(i%2) .. 8*(i%2)+8.  Each core computes its
QKV projection slice, RoPE, causal attention for its 8 heads, and a partial
output projection over its 512 d-columns; the host sums the two partials per
batch.

On-core dataflow (per core, all matmul operands bf16, psum/softmax fp32):
  qkT[e,l] = Wqk_sub @ x^T       (e = 8 q-heads then 8 k-heads, dh-major)
  rope on qkT rows (pair-swap via stream_shuffle + cos/sin tables)
  V[l,e]   = x @ Wv_sub^T        (natural orientation, 8 heads * 64)
  per head-pair (2 heads stacked in one 128-partition chunk):
    S^T[k,q] = K^T Q  (row-paired K=64 matmuls, tile_position (0,0)/(64,0))
    P^T = exp(S^T/8) with causal masking (valid-range exp + tri-mask)
    s[q] = colsum(P^T) broadcast via ones-matmul (col-paired (0,0)/(0,64))
    O^T[dh,q] = V^T-matmul accumulation (col-paired)
    O^T /= s
  y[l,e] partial = O^T.T @ Wo_sub  (lhsT = O^T chunks)

Scheduling: single fused phase.  Rounds are ordered jq-major (q-block outer,
head-pair inner) and the QKV-projection l4-blocks, V tiles, and output-
projection halves are emitted just-in-time / interleaved between attention
tiles so the tensor engine never idles and exp (scalar engine) overlaps the
projection work.  The exp input is packed (b-half shifted down by vs) so no
garbage columns are exponentiated.
"""
import sys
sys.path.insert(0, "/opt/trn_rl_repo")

import numpy as np
import ml_dtypes

B, L, D, H = 4, 2048, 1024, 16
DH = D // H  # 64
THETA = 100000.0
NCORES = 8
BF = ml_dtypes.bfloat16

_built = None


def _rope_tables():
    # [128, L] f32: rows = 2 stacked heads' dh (64 each), identical per head.
    pos = np.arange(L, dtype=np.float32)
    inv_freq = (1.0 / THETA ** (np.arange(0, DH, 2, dtype=np.float32) / DH))
    ang = pos[None, :] * inv_freq[:, None]              # [32, L]
    cos = np.cos(ang)                                    # [32, L]
    sin = np.sin(ang)
    cos2 = np.repeat(cos, 2, axis=0)                     # rows 2p,2p+1 = cos_p
    sin2 = np.empty((DH, L), np.float32)
    sin2[0::2] = -sin
    sin2[1::2] = sin
    return (np.concatenate([cos2, cos2], 0).astype(np.float32),
            np.concatenate([sin2, sin2], 0).astype(np.float32))


def _build():
    import concourse.mybir as mybir
    import concourse.tile as tile
    from concourse import bacc

    FP32 = mybir.dt.float32
    BF16 = mybir.dt.bfloat16
    MUL = mybir.AluOpType.mult
    ADD = mybir.AluOpType.add
    EXP = mybir.ActivationFunctionType.Exp
    SWAP_MASK = [i ^ 1 for i in range(32)]
    SKEW = 3

    nc = bacc.Bacc(None, target_bir_lowering=False)
    # DRAM parameters (per-core shapes; host prepares layouts)
    xt_d = nc.declare_dram_parameter("xt", [128, 8, L], BF16, False)        # x^T, partition-major
    wqk_d = nc.declare_dram_parameter("wqk", [8, 128, 8, 128], BF16, False)  # [echunk, d, dchunk, e]
    wv_d = nc.declare_dram_parameter("wv", [128, 8, 512], BF16, False)      # [d, dchunk, e_v]
    wo_d = nc.declare_dram_parameter("wo", [128, 8, 512], BF16, False)      # [d, dchunk*ehalf, e]
    cos_d = nc.declare_dram_parameter("cos2", [128, L], FP32, False)
    sin_d = nc.declare_dram_parameter("sin2", [128, L], FP32, False)
    tri_d = nc.declare_dram_parameter("trimask", [128, 128], BF16, False)
    y_d = nc.declare_dram_parameter("y", [L, D], FP32, True)

    with tile.TileContext(nc) as tc:
        import contextlib
        ctx = contextlib.ExitStack()
        with ctx:
            # ---- SBUF pools ----
            res = ctx.enter_context(tc.tile_pool(name="res", bufs=1))
            wq_pool = ctx.enter_context(tc.tile_pool(name="wqk", bufs=8))
            rope_pool = ctx.enter_context(tc.tile_pool(name="rope", bufs=3))
            pt_pool = ctx.enter_context(tc.tile_pool(name="pt", bufs=6))
            rec_pool = ctx.enter_context(tc.tile_pool(name="rec", bufs=1))
            y_pool = ctx.enter_context(tc.tile_pool(name="yt", bufs=2))
            # ---- PSUM pools (8 banks total, all open for whole kernel) ----
            st_ps = ctx.enter_context(
                tc.tile_pool(name="ps_st", bufs=2, space="PSUM"))   # 4 banks
            av_ps = ctx.enter_context(
                tc.tile_pool(name="ps_av", bufs=1, space="PSUM"))   # 2 banks
            sh_ps = ctx.enter_context(
                tc.tile_pool(name="ps_sh", bufs=2, space="PSUM"))   # 2 banks

            xt_all = res.tile([128, 8 * L], BF16, tag="xt", name="xt")
            xt = [xt_all[:, d * L:(d + 1) * L] for d in range(8)]
            qkr = [res.tile([128, L], BF16, tag=f"qkr{c}", name=f"qkr{c}")
                   for c in range(8)]
            # V tiles with the softmax-denominator trick: per head h the
            # 128-col block is [ones (64 cols) | V_h (64 cols)], so the AV
            # matmul's output rows 0:64 are the colsum of P^T for free
            # (ones first so the reciprocal reads a base-0 partition range).
            vsb = [res.tile([128, 1024], BF16, tag=f"v{t}", name=f"v{t}")
                   for t in range(16)]
            wv_all = res.tile([128, 4096], BF16, tag="wv", name="wv")
            wv_sb = [wv_all[:, d * 512:(d + 1) * 512] for d in range(8)]
            wo_all = res.tile([128, 4096], BF16, tag="wo", name="wo")
            wo_sb = [wo_all[:, i * 512:(i + 1) * 512] for i in range(8)]
            cos_sb = res.tile([128, L], FP32, tag="cos")
            sin_sb = res.tile([128, L], FP32, tag="sin")
            tri_sb = res.tile([128, 128], BF16, tag="tri")
            ones_sb = res.tile([128, DH], BF16, tag="ones")
            wsc_sb = res.tile([128, 512], BF16, tag="wsc")
            ot = [res.tile([128, L], BF16, tag=f"ot{p}", name=f"ot{p}")
                  for p in range(4)]

            # ---- PE warmup: ~4.5us of matmuls with no DMA dependency so the
            # HAM clock-gate opens while the prologue DMAs are in flight ----
            nc.vector.memset(ones_sb, 1.0)
            nc.vector.memset(wsc_sb, 0.0)
            wu = av_ps.tile([128, 512], FP32, tag="ava", name="wu")
            for i in range(40):
                nc.tensor.matmul(wu[0:64, 0:128], ones_sb, wsc_sb[:, 0:128],
                                 start=True, stop=True)

            # ---- JIT DMA/emission state ----
            w_chunks = {}
            xt_q = set()
            cs_q = set()
            blocks_done = set()
            v_done = set()
            wo_loaded = [False]

            def ensure_xt(l4):
                if l4 in xt_q:
                    return
                xt_q.add(l4)
                lsl = slice(512 * l4, 512 * l4 + 512)
                dst = xt_all[:, 0:8 * L].rearrange(
                    "p (d l) -> p d l", l=L)[:, :, lsl]
                nc.sync.dma_start(out=dst, in_=xt_d[:, :, lsl])

            def ensure_cs(l4):
                if l4 in cs_q:
                    return
                cs_q.add(l4)
                lsl = slice(512 * l4, 512 * l4 + 512)
                nc.sync.dma_start(out=cos_sb[:, lsl], in_=cos_d[:, lsl])
                nc.sync.dma_start(out=sin_sb[:, lsl], in_=sin_d[:, lsl])

            def ensure_w(c):
                if c not in w_chunks:
                    wt = wq_pool.tile([128, 1024], BF16, tag="w",
                                      name=f"w_{c}")
                    nc.sync.dma_start(out=wt, in_=wqk_d[c])
                    w_chunks[c] = [wt[:, 128 * d:128 * d + 128]
                                   for d in range(8)]
                return w_chunks[c]

            def emit_block(c, l4):
                """One 512-wide l-block of QKV-projection chunk c (+RoPE)."""
                if (c, l4) in blocks_done:
                    return
                blocks_done.add((c, l4))
                wts = ensure_w(c)
                ensure_xt(l4)
                ensure_cs(l4)
                lsl = slice(512 * l4, 512 * l4 + 512)
                qkp = sh_ps.tile([128, 512], FP32, tag="ps")
                for d in range(8):
                    nc.tensor.matmul(qkp, wts[d], xt[d][:, lsl],
                                     start=(d == 0), stop=(d == 7))
                shf = rope_pool.tile([128, 512], FP32, tag="shf")
                nc.vector.stream_shuffle(shf, qkp, SWAP_MASK)
                t1 = rope_pool.tile([128, 512], FP32, tag="t1")
                nc.vector.tensor_tensor(out=t1, in0=qkp, in1=cos_sb[:, lsl],
                                        op=MUL)
                t2 = rope_pool.tile([128, 512], FP32, tag="t2")
                nc.gpsimd.tensor_tensor(out=t2, in0=shf, in1=sin_sb[:, lsl],
                                        op=MUL)
                nc.gpsimd.tensor_tensor(out=qkr[c][:, lsl], in0=t1, in1=t2,
                                        op=ADD)

            def emit_v(t):
                if t in v_done:
                    return
                v_done.add(t)
                ensure_xt(t // 4)
                vp = sh_ps.tile([128, 512], FP32, tag="ps")
                lsl = slice(128 * t, 128 * t + 128)
                for d in range(8):
                    nc.tensor.matmul(vp, xt[d][:, lsl], wv_sb[d],
                                     start=(d == 0), stop=(d == 7))
                # scatter the 8 heads' 64 V columns into the [V_h | ones]
                # blocks (ones halves were memset in the prologue)
                dst = vsb[t][:, 0:1024].rearrange(
                    "p (h c) -> p h c", c=128)[:, :, 64:128]
                src = vp[:, 0:512].rearrange("p (h c) -> p h c", c=64)
                nc.vector.tensor_copy(out=dst, in_=src)

            def emit_proj(t, eh):
                lsl = slice(128 * t, 128 * t + 128)
                yp = sh_ps.tile([128, 512], FP32, tag="ps")
                for dc in range(4):
                    nc.tensor.matmul(yp, ot[dc][:, lsl], wo_sb[dc * 2 + eh],
                                     start=(dc == 0), stop=(dc == 3))
                yt = y_pool.tile([128, 512], FP32, tag="yt")
                nc.vector.tensor_copy(out=yt, in_=yp)
                nc.sync.dma_start(out=y_d[lsl, 512 * eh:512 * eh + 512],
                                  in_=yt)

            def emit_wo():
                if wo_loaded[0]:
                    return
                wo_loaded[0] = True
                nc.sync.dma_start(out=wo_all, in_=wo_d[:, :, :])

            # ---- prologue DMAs + first round's blocks ----
            emit_block(0, 0)
            emit_block(4, 0)
            nc.sync.dma_start(out=tri_sb, in_=tri_d[:, :])
            nc.sync.dma_start(out=wv_all, in_=wv_d[:, :, :])
            for t in range(16):
                ones_half = vsb[t][:, 0:1024].rearrange(
                    "p (h c) -> p h c", c=128)[:, :, 0:64]
                nc.vector.memset(ones_half, 1.0)

            # ---- fused main loop: attention rounds, jq-major ----
            rounds = [(jq, p) for jq in range(4) for p in range(4)]
            proj_queue = []
            pending_norm = [None]

            def emit_norm(p, qb0, ava, avb, n):
                rsa = rec_pool.tile([64, 512], FP32, tag="rsa", name=f"ra{n}")
                rsb = rec_pool.tile([64, 512], FP32, tag="rsb", name=f"rb{n}")
                nc.vector.reciprocal_approx_fast(out=rsa, in_=ava[0:64])
                nc.vector.reciprocal_approx_fast(out=rsb, in_=avb[0:64])
                nc.vector.tensor_tensor(out=ot[p][0:64, qb0:qb0 + 512],
                                        in0=ava[64:128], in1=rsa, op=MUL)
                nc.vector.tensor_tensor(out=ot[p][64:128, qb0:qb0 + 512],
                                        in0=avb[64:128], in1=rsb, op=MUL)

            for n, (jq, p) in enumerate(rounds):
                qb0 = 512 * jq
                nk = 4 * (jq + 1)
                emit_block(p, jq)
                emit_block(4 + p, jq)
                ava = av_ps.tile([128, 512], FP32, tag="ava", name=f"ava{n}")
                avb = av_ps.tile([128, 512], FP32, tag="avb", name=f"avb{n}")
                qt, kt = qkr[p], qkr[4 + p]
                vca, vcb = 256 * p, 256 * p + 128

                def sav(k, pt, vs):
                    first, last = (k == 0), (k == nk - 1)
                    isl = slice(vs, 512)
                    bsl = slice(512, 1024 - vs)
                    nc.tensor.matmul(ava[:, isl],
                                     vsb[k][:, vca:vca + 128], pt[:, isl],
                                     start=first, stop=last,
                                     skip_group_check=True)
                    nc.tensor.matmul(avb[:, isl],
                                     vsb[k][:, vcb:vcb + 128], pt[:, bsl],
                                     start=first, stop=last,
                                     skip_group_check=True)

                if n + 1 < len(rounds):
                    jq2, p2 = rounds[n + 1]
                    next_blocks = [(p2, jq2), (4 + p2, jq2)]
                else:
                    next_blocks = []

                pending = []
                for k in range(nk):
                    # interleaved filler work (before S^T so an ACT-lag stall
                    # on the st ring doesn't starve the tensor engine)
                    if k == 1 and pending_norm[0] is not None:
                        pending_norm[0]()
                        pending_norm[0] = None
                    if p == 0 and k < 4:
                        emit_v(4 * jq + k)
                    if k == 0 and p == 1 and jq < 3:
                        ensure_xt(jq + 1)
                        ensure_cs(jq + 1)
                    if k == 1 and next_blocks:
                        emit_block(*next_blocks[0])
                    if k == 3 and next_blocks:
                        emit_block(*next_blocks[1])
                    if jq == 0 and p == 3 and k == 3:
                        emit_wo()
                    if k % 4 == 3 and proj_queue:
                        emit_proj(*proj_queue.pop(0))
                    kpos = 128 * k
                    vs = max(0, kpos - qb0)
                    st = st_ps.tile([128, 1024], FP32, tag="st")
                    ksl = slice(kpos, kpos + 128)
                    qsl = slice(qb0 + vs, qb0 + 512)
                    nc.tensor.matmul(st[:, vs:512], kt[0:64, ksl],
                                     qt[0:64, qsl], start=True, stop=True,
                                     tile_position=(0, 0))
                    nc.tensor.matmul(st[:, 512:1024 - vs], kt[64:128, ksl],
                                     qt[64:128, qsl], start=True, stop=True,
                                     tile_position=(64, 0))
                    pt = pt_pool.tile([128, 1024], BF16, tag="pt")
                    nc.scalar.activation(out=pt[:, vs:1024 - vs],
                                         in_=st[:, vs:1024 - vs],
                                         func=EXP, scale=0.125)
                    if kpos >= qb0:
                        nc.vector.tensor_tensor(out=pt[:, vs:vs + 128],
                                                in0=pt[:, vs:vs + 128],
                                                in1=tri_sb, op=MUL)
                        nc.vector.tensor_tensor(out=pt[:, 512:640],
                                                in0=pt[:, 512:640],
                                                in1=tri_sb, op=MUL)
                    pending.append((k, pt, vs))
                    if len(pending) > SKEW:
                        sav(*pending.pop(0))
                for args in pending:
                    sav(*args)
                pending_norm[0] = (lambda p=p, qb0=qb0, ava=ava, avb=avb, n=n:
                                   emit_norm(p, qb0, ava, avb, n))
                if p == 3:
                    for t in range(4 * jq, 4 * jq + 4):
                        for eh in range(2):
                            proj_queue.append((t, eh))

            pending_norm[0]()
            pending_norm[0] = None
            for t, eh in proj_queue:
                emit_proj(t, eh)
    nc.compile()
    return nc


def _get_nc():
    global _built
    if _built is None:
        _built = _build()
    return _built


def _in_maps(x, W, Wo):
    x = np.asarray(x, np.float32)
    W = np.asarray(W, np.float32)
    Wo = np.asarray(Wo, np.float32)

    cos2, sin2 = _rope_tables()
    tri = np.zeros((128, 128), np.float32)
    p_idx = np.arange(128)
    tri[p_idx[:, None] <= p_idx[None, :]] = 1.0  # valid: k <= q
    tri = tri.astype(BF)

    in_maps = []
    for core in range(NCORES):
        b, g = core // 2, core % 2
        xt = np.ascontiguousarray(x[b].T).astype(BF)                # [D, L]
        xt = np.ascontiguousarray(
            xt.reshape(8, 128, L).transpose(1, 0, 2))               # [128,8,L]
        wq = W[512 * g:512 * g + 512]                                # [512, D]
        wk = W[D + 512 * g:D + 512 * g + 512]
        wv = W[2 * D + 512 * g:2 * D + 512 * g + 512]
        wqk_t = np.ascontiguousarray(
            np.concatenate([wq, wk], 0).T).astype(BF)                # [D, 1024]
        # -> [echunk, 128(d), dchunk, 128(e)]
        wqk_t = wqk_t.reshape(8, 128, 8, 128).transpose(2, 1, 0, 3)
        wqk_t = np.ascontiguousarray(wqk_t)
        wv_t = np.ascontiguousarray(
            wv.T.astype(BF).reshape(8, 128, 512).transpose(1, 0, 2))
        wo_t = np.ascontiguousarray(Wo[:, 512 * g:512 * g + 512].T).astype(BF)  # [512, D]
        wo_t = np.ascontiguousarray(
            wo_t.reshape(4, 128, 2, 512).transpose(1, 0, 2, 3)
            .reshape(128, 8, 512))
        in_maps.append({
            "xt": xt, "wqk": wqk_t, "wv": wv_t, "wo": wo_t,
            "cos2": cos2, "sin2": sin2, "trimask": tri,
        })
    return in_maps


def kernel(x, W, Wo):
    from concourse.bass_utils import run_bass_kernel_spmd

    res = run_bass_kernel_spmd(_get_nc(), _in_maps(x, W, Wo),
                               list(range(NCORES)))
    out = np.empty((B, L, D), np.float32)
    for b in range(B):
        out[b] = res.results[2 * b]["y"] + res.results[2 * b + 1]["y"]
    return out


def _install_ntff_hook_shim():
    """The trimmed repo lacks antenv.axon_hooks; reconstruct it so
    run_bass_kernel_spmd(trace=True) can NTFF-profile through axon."""
    import sys as _sys, types
    if "antenv.axon_hooks" in _sys.modules:
        return
    import antenv  # noqa: F401
    from trn_agent_boot.trn_boot import _ntff_profile_via_ctypes
    hook = _ntff_profile_via_ctypes("/opt/axon/libaxon_pjrt.so")
    mod = types.ModuleType("antenv.axon_hooks")
    mod.set_axon_ntff_profile_hook = lambda h: None
    mod.get_axon_ntff_profile_hook = lambda: hook
    _sys.modules["antenv.axon_hooks"] = mod


def kernel_traced(x, W, Wo, tmpdir=None):
    """Run with NTFF tracing; returns BassKernelResults (trace in tmpdir)."""
    from concourse.bass_utils import run_bass_kernel_spmd

    _install_ntff_hook_shim()
    res = run_bass_kernel_spmd(_get_nc(), _in_maps(x, W, Wo),
                               list(range(NCORES)), trace=True, tmpdir=tmpdir)
    return res.exec_time_ns
